# revision 1
# baseline (speedup 1.0000x reference)
"""BertSelfAttention (B=4, S=2048, H=768, 12 heads) on 8 TRN2 NeuronCores.

Sharding: core c -> (batch b = c//2, head-group g = c%2).  Each core computes
6 heads of one batch: Q/K/V projections restricted to that head group's 384
columns of Wq/Wk/Wv, the [S, S] score block per head, softmax, and the
context.  No cross-core communication.

Per-core dataflow (all matmuls bf16 in / f32 PSUM accumulate):
  X^T   : gpsimd cast-DMA f32->bf16 into DRAM scratch, then XBAR
          transpose-load -> SBUF [128d, 2048s] x 6
  Q^T,K^T: lhsT=W tile [d,e], rhs=X^T -> PSUM [e,s]; DVE copy + bias -> bf16
  V      : lhsT=X^T tile [d,s], rhs=Wv -> PSUM [s,e]; DVE copy + bias -> bf16
           stored per head with an extra all-ones column ([V_h | 1], 65 cols)
  scores^T: lhsT=K^T_h [64,128k], rhs=Q^T_h [64,512q] -> PSUM [128k, q]
            (head pairs packed into rows 0-63 / 64-127 of the PE array)
  E^T    : ScalarE exp(0.125*s + mask_k) PSUM->SBUF bf16  (mask is a
           per-partition bias in this orientation; denominator scaling is
           deferred)
  ctx    : lhsT=[V_h|1] [128k, 65], rhs=E^T -> PSUM [65, q] accumulated over
           16 k-tiles; row 64 is the softmax denominator
  out    : DVE reciprocal(denom) -> gpsimd partition_broadcast -> DVE mult
           -> SBUF f32 [64, 2048] per head -> DMA to DRAM out^T [384, 2048]

The host transposes each core's [384, 2048] back to [2048, 384] during the
gather (pure layout).
"""

import sys

sys.path.insert(0, "/opt/trn_rl_repo")

import numpy as np

B = 4
S = 2048
HIDDEN = 768
HEADS = 12
DHEAD = 64
NCORES = 8
HLOC = 6            # heads per core
ELOC = HLOC * DHEAD  # 384 embedding columns per core
P = 128
NDT = HIDDEN // P   # 6 d-tiles (contraction)
NET = ELOC // P     # 3 e-tiles
NKT = S // P        # 16 k-tiles
QH = 1024           # q-half width (exp granularity; 2 PSUM banks)
NQH = S // QH       # 2

_CACHE = {}


def _emit(tc, aps):
    """Emit the per-core program into TileContext tc."""
    import concourse.bass as bass
    from concourse import mybir
    from concourse.masks import make_identity

    from contextlib import ExitStack

    nc = tc.nc
    f32 = mybir.dt.float32
    bf16 = mybir.dt.bfloat16
    Exp = mybir.ActivationFunctionType.Exp
    ts = bass.ts
    QQ = 512                 # q-quarter width
    NQQ = S // QQ            # 4

    x, wq, wk, wv, bq, bk, bv, mask, out = (
        aps["x"], aps["wq"], aps["wk"], aps["wv"],
        aps["bq"], aps["bk"], aps["bv"], aps["mask"], aps["out"],
    )

    stack = ExitStack()
    persist = stack.enter_context(tc.tile_pool(name="persist", bufs=1))
    dram = stack.enter_context(tc.tile_pool(name="dram", bufs=1, space="DRAM"))
    sc_pool = stack.enter_context(tc.tile_pool(name="sc", bufs=2, space="PSUM"))
    ctx_pool = stack.enter_context(tc.tile_pool(name="ctx", bufs=4, space="PSUM"))
    et_pool = stack.enter_context(tc.tile_pool(name="et", bufs=6))
    r_pool = stack.enter_context(tc.tile_pool(name="r", bufs=3))
    r0_pool = stack.enter_context(tc.tile_pool(name="r0", bufs=3))
    rbc_pool = stack.enter_context(tc.tile_pool(name="rbc", bufs=3))
    oh_pool = stack.enter_context(tc.tile_pool(name="oh", bufs=4))

    # ---- startup DMA plan: sync loads W (f32, no cast) while gpsimd casts X
    # to bf16 in two column halves; XBAR transpose-loads follow the casts on
    # sync with no interleaved DMACopy (mode flips serialize the XBAR).
    # W is cast f32->bf16 on the idle DVE.
    w_f32 = {}
    for name, w in (("q", wq), ("k", wk), ("v", wv)):
        tf = persist.tile([P, NDT, ELOC], f32, tag=f"wf{name}", name=f"wf{name}")
        nc.sync.dma_start(out=tf[:], in_=w.rearrange("(t p) e -> p t e", p=P))
        w_f32[name] = tf

    xt = persist.tile([P, NDT, S], bf16, tag="xt")
    xbfs = []
    for half in range(2):
        cols = slice(half * (HIDDEN // 2), (half + 1) * (HIDDEN // 2))
        xbf = dram.tile([S, HIDDEN // 2], bf16, tag="xbf", name=f"xbf{half}")
        nc.gpsimd.dma_start(out=xbf[:], in_=x[:, cols])      # cast f32->bf16
        xbfs.append(xbf)
    for half in range(2):
        for jj in range(NDT // 2):
            j = half * (NDT // 2) + jj
            nc.sync.dma_start(out=xt[:, j, :], in_=xbfs[half][:, ts(jj, P)], transpose=True)

    w_sb = {}
    for name in ("q", "k", "v"):
        t = persist.tile([P, NDT, ELOC], bf16, tag=f"w{name}", name=f"w{name}")
        nc.vector.tensor_copy(t[:], w_f32[name][:])
        w_sb[name] = t

    # ---- mask/bq/bk: load as rows (contiguous, descriptor-light, SWDGE),
    # then one PE transpose into per-partition layout.
    combo = persist.tile([32, P], f32, tag="combo")
    nc.vector.memset(combo[:], 0.0)
    nc.gpsimd.dma_start(out=combo[0:NKT, :], in_=mask.rearrange("(t p) -> t p", p=P))
    nc.gpsimd.dma_start(out=combo[NKT : NKT + NET, :], in_=bq.rearrange("(t p) -> t p", p=P))
    nc.gpsimd.dma_start(out=combo[NKT + NET : NKT + 2 * NET, :], in_=bk.rearrange("(t p) -> t p", p=P))
    ident = persist.tile([32, 32], f32, tag="ident")
    make_identity(nc, ident[:])
    const_ps = sc_pool.tile([P, 32], f32, tag="sc", name="constps")
    nc.tensor.transpose(const_ps[:], combo[:], ident[:])
    const_sb = persist.tile([P, 32], f32, tag="const")
    nc.vector.tensor_copy(const_sb[:], const_ps[:])
    mask_sb = const_sb[:, 0:NKT]
    bq_sb = const_sb[:, NKT : NKT + NET]
    bk_sb = const_sb[:, NKT + NET : NKT + 2 * NET]

    bv_row = persist.tile([1, ELOC], f32, tag="bvr")
    nc.gpsimd.dma_start(out=bv_row[:], in_=bv[None, :])
    bv_bc = persist.tile([P, ELOC], f32, tag="bvb")
    nc.gpsimd.partition_broadcast(bv_bc[:], bv_row[:])

    # ---- V projection: V[s, e] = X @ Wv + bv, stored [128s, 6h, 65] bf16 ----
    v_sb = persist.tile([P, NKT, HLOC, DHEAD + 1], bf16, tag="v")

    def emit_v():
        for st in range(NKT):
            nc.vector.memset(v_sb[:, st, :, DHEAD:], 1.0)  # ones column
        for st in range(NKT):
            vps = ctx_pool.tile([P, ELOC], f32, tag="ctx", name=f"vps{st}")
            for dt_i in range(NDT):
                nc.tensor.matmul(
                    vps[:],
                    lhsT=xt[:, dt_i, ts(st, P)],
                    rhs=w_sb["v"][:, dt_i, :],
                    start=(dt_i == 0),
                    stop=(dt_i == NDT - 1),
                )
            nc.vector.tensor_add(
                v_sb[:, st, :, 0:DHEAD],
                vps[:].rearrange("p (h d) -> p h d", d=DHEAD),
                bv_bc[:].rearrange("p (h d) -> p h d", d=DHEAD),
            )

    # ---- Q^T / K^T projections: [e, s] = W.T @ X^T + b ----
    qt_sb = persist.tile([P, NET, S], bf16, tag="qt")
    kt_sb = persist.tile([P, NET, S], bf16, tag="kt")

    def qk_group(proj, et_i, sb_i):
        dst, b_sb = (qt_sb, bq_sb) if proj == "q" else (kt_sb, bk_sb)
        qps = ctx_pool.tile([P, 512], f32, tag="ctx", name=f"qps{proj}{et_i}_{sb_i}")
        for dt_i in range(NDT):
            nc.tensor.matmul(
                qps[:],
                lhsT=w_sb[proj][:, dt_i, ts(et_i, P)],
                rhs=xt[:, dt_i, ts(sb_i, 512)],
                start=(dt_i == 0),
                stop=(dt_i == NDT - 1),
            )
        nc.vector.tensor_scalar_add(
            dst[:, et_i, ts(sb_i, 512)], qps[:], b_sb[:, et_i : et_i + 1]
        )

    def emit_qk(et_i):
        for proj in ("q", "k"):
            for sb_i in range(S // 512):
                qk_group(proj, et_i, sb_i)

    # ---- attention ----
    # q-quarter structure: both heads' scores for one k-tile live in ONE PSUM
    # tile [128, 2, 512] so the pair of score matmuls has no semaphore wait
    # between them (they pack into array rows 0-63 / 64-127 concurrently via
    # tile_position) and one exp covers both heads ([128, 1024]).
    # ctx lags one k-tile behind so PE never stalls on the current exp.
    # Projections for the next head pair are drip-fed between k-tiles.
    emit_qk(0)
    emit_v()
    for pair in range(NET):  # e-tile == head pair
        fillers = []
        if pair + 1 < NET:
            fillers = [
                (lambda p=proj, s=sb_i: qk_group(p, pair + 1, s))
                for proj in ("q", "k")
                for sb_i in range(S // 512)
            ]
        ohs = [oh_pool.tile([DHEAD, S], f32, tag="oh", name=f"oh{pair}_{i}") for i in range(2)]
        it = 0
        for qq in range(NQQ):
            ctx_ps = [
                ctx_pool.tile([DHEAD + 1, QQ], f32, tag="ctx", name=f"ctx{pair}_{qq}_{i}")
                for i in range(2)
            ]

            def emit_ctx(t, et_t):
                for hl in range(2):
                    nc.tensor.matmul(
                        ctx_ps[hl][:],
                        lhsT=v_sb[:, t, 2 * pair + hl, :],
                        rhs=et_t[:, hl, :],
                        start=(t == 0),
                        stop=(t == NKT - 1),
                    )

            prev = None
            for t in range(NKT):
                s_t = sc_pool.tile([P, 2, QQ], f32, tag="sc", name=f"s{pair}_{qq}_{t}")
                for hl in range(2):
                    rows = slice(DHEAD * hl, DHEAD * (hl + 1))
                    nc.tensor.matmul(
                        s_t[:, hl, :],
                        lhsT=kt_sb[rows, pair, ts(t, P)],
                        rhs=qt_sb[rows, pair, ts(qq, QQ)],
                        start=True,
                        stop=True,
                        tile_position=(DHEAD * hl, 0),
                    )
                et_t = et_pool.tile([P, 2, QQ], bf16, tag="et", name=f"et{pair}_{qq}_{t}")
                nc.scalar.activation(
                    et_t[:], s_t[:], Exp,
                    bias=mask_sb[:, t : t + 1], scale=0.125,
                )
                if prev is not None:
                    emit_ctx(t - 1, prev)
                prev = et_t
                it += 1
                if fillers and it % 6 == 5:
                    fillers.pop(0)()
            emit_ctx(NKT - 1, prev)

            for hl in range(2):
                # Evacuate ctx+denom to SBUF right away (frees the PSUM slot),
                # then normalize from SBUF.  custom-DVE/gpsimd ops need base
                # partition 0 on HW, so the denom row is DMA-hopped first.
                ctx_sb = r_pool.tile([DHEAD + 1, QQ], f32, tag="r")
                nc.vector.tensor_copy(ctx_sb[:], ctx_ps[hl][:])
                r0 = r0_pool.tile([1, QQ], f32, tag="r0")
                nc.sync.dma_start(out=r0[:], in_=ctx_sb[DHEAD : DHEAD + 1, :])
                rr = r0_pool.tile([1, QQ], f32, tag="rr")
                nc.vector.reciprocal_approx_fast(rr[:], r0[:])
                rbc = rbc_pool.tile([DHEAD, QQ], f32, tag="rbc")
                nc.gpsimd.partition_broadcast(rbc[:], rr[:])
                nc.vector.tensor_mul(
                    ohs[hl][:, ts(qq, QQ)], ctx_sb[0:DHEAD, :], rbc[:]
                )
                nc.sync.dma_start(
                    out=out[ts(2 * pair + hl, DHEAD), ts(qq, QQ)],
                    in_=ohs[hl][:, ts(qq, QQ)],
                )
        while fillers:
            fillers.pop(0)()

    stack.close()


def build():
    """Build and compile the per-core Bass program (same program on all 8 cores)."""
    if "nc" in _CACHE:
        return _CACHE["nc"]
    import concourse.bass as bass  # noqa: F401
    import concourse.tile as tile
    from concourse import bacc, mybir

    f32 = mybir.dt.float32
    nc = bacc.Bacc("TRN2", target_bir_lowering=False, debug=False, num_devices=NCORES)
    aps = {
        "x": nc.dram_tensor("x", [S, HIDDEN], f32, kind="ExternalInput").ap(),
        "wq": nc.dram_tensor("wq", [HIDDEN, ELOC], f32, kind="ExternalInput").ap(),
        "wk": nc.dram_tensor("wk", [HIDDEN, ELOC], f32, kind="ExternalInput").ap(),
        "wv": nc.dram_tensor("wv", [HIDDEN, ELOC], f32, kind="ExternalInput").ap(),
        "bq": nc.dram_tensor("bq", [ELOC], f32, kind="ExternalInput").ap(),
        "bk": nc.dram_tensor("bk", [ELOC], f32, kind="ExternalInput").ap(),
        "bv": nc.dram_tensor("bv", [ELOC], f32, kind="ExternalInput").ap(),
        "mask": nc.dram_tensor("mask", [S], f32, kind="ExternalInput").ap(),
        "out": nc.dram_tensor("out", [ELOC, S], f32, kind="ExternalOutput").ap(),
    }
    with tile.TileContext(nc) as tc:
        _emit(tc, aps)
    nc.compile()
    _CACHE["nc"] = nc
    return nc


def shard_inputs(hidden_states, attention_mask, Wq, bq, Wk, bk, Wv, bv):
    in_maps = []
    for c in range(NCORES):
        b, g = divmod(c, 2)
        cols = slice(ELOC * g, ELOC * (g + 1))
        in_maps.append({
            "x": np.ascontiguousarray(hidden_states[b], dtype=np.float32),
            "wq": np.ascontiguousarray(Wq[:, cols], dtype=np.float32),
            "wk": np.ascontiguousarray(Wk[:, cols], dtype=np.float32),
            "wv": np.ascontiguousarray(Wv[:, cols], dtype=np.float32),
            "bq": np.ascontiguousarray(bq[cols], dtype=np.float32),
            "bk": np.ascontiguousarray(bk[cols], dtype=np.float32),
            "bv": np.ascontiguousarray(bv[cols], dtype=np.float32),
            "mask": np.ascontiguousarray(
                np.asarray(attention_mask, dtype=np.float32)[b].reshape(S)
            ),
        })
    return in_maps


def gather_outputs(results):
    out = np.empty((B, S, HIDDEN), dtype=np.float32)
    for c in range(NCORES):
        b, g = divmod(c, 2)
        out[b, :, ELOC * g : ELOC * (g + 1)] = np.ascontiguousarray(results[c]["out"].T)
    return out


def kernel(**inputs):
    from concourse.bass_utils import run_bass_kernel_spmd

    nc = build()
    in_maps = shard_inputs(**{k: np.asarray(v) for k, v in inputs.items()})
    res = run_bass_kernel_spmd(nc, in_maps, list(range(NCORES)))
    return gather_outputs(res.results)


if __name__ == "__main__":
    nc = build()
    print("build + compile OK")



# revision 5
# speedup vs baseline: 1.0634x; 1.0634x over previous
"""BertSelfAttention (B=4, S=2048, H=768, 12 heads) on 8 TRN2 NeuronCores.

Sharding: core c -> (batch b = c//2, head-group g = c%2).  Each core computes
6 heads of one batch: Q/K/V projections restricted to that head group's 384
columns of Wq/Wk/Wv, the [S, S] score block per head, softmax, and the
context.  No cross-core communication.

Steady state is Scalar-engine limited (one exp ACTIVATE [128k, 2h, 512q] per
k-tile iteration, 192 iterations).  The schedule is a single flat stream of
192 (pair, qq, k-tile) iterations; everything else (input transposes, Q/K/V
projections) is drip-fed into the stream's PE slack so the Scalar engine
starts exp'ing at ~9 us and never starves:

  X input : 16 row-chunk DMAs [128s, 768d] f32 (contiguous), PE-transposed
            per 128x128 tile into PSUM, copy-cast f32->bf16 to SBUF X^T
            (Scalar does the first chunks' copies, DVE the rest)
  W       : f32 DMA (pair-0 e-tile of Wq/Wk first), DVE cast to bf16
  Q^T,K^T : lhsT=W tile [d,e], rhs=X^T -> PSUM [e,s]; DVE copy + bias -> bf16
  V       : lhsT=X^T tile [d,s], rhs=Wv -> PSUM [s,e]; DVE copy + bias -> bf16
            stored per head with an extra all-ones column ([V_h | 1], 65 cols)
  scores^T: lhsT=K^T_h [64,128k], rhs=Q^T_h [64,512q] -> PSUM [128k, 2, 512]
            (head pairs pack into PE rows 0-63 / 64-127 via tile_position and
            stream concurrently)
  E^T     : ScalarE exp(0.125*s + mask_k) PSUM->SBUF bf16 [128, 1024]
  ctx     : lhsT=[V_h|1] [128k, 65], rhs=E^T -> PSUM [65, q], accumulated over
            16 k-tiles; row 64 is the softmax denominator.  ctx runs at lag 2
            behind exp so the PE never waits on the current ACTIVATE.
  out     : DVE evacuate + reciprocal(denom) -> gpsimd partition_broadcast ->
            DVE mult -> SBUF f32 [64, 2048] per head -> DMA to DRAM out^T

The host transposes each core's [384, 2048] back to [2048, 384] during the
gather (pure layout).
"""

import sys

sys.path.insert(0, "/opt/trn_rl_repo")

import numpy as np

B = 4
S = 2048
HIDDEN = 768
HEADS = 12
DHEAD = 64
NCORES = 8
HLOC = 6            # heads per core
ELOC = HLOC * DHEAD  # 384 embedding columns per core
P = 128
NDT = HIDDEN // P   # 6 d-tiles (contraction)
NET = ELOC // P     # 3 e-tiles (head pairs)
NKT = S // P        # 16 k-tiles
NCH = S // P        # 16 s-chunks of X rows

_CACHE = {}


def _emit(tc, aps):
    """Emit the per-core program into TileContext tc."""
    import concourse.bass as bass
    from concourse import mybir
    from concourse.masks import make_identity

    from contextlib import ExitStack

    nc = tc.nc
    f32 = mybir.dt.float32
    bf16 = mybir.dt.bfloat16
    Exp = mybir.ActivationFunctionType.Exp
    ts = bass.ts
    QQ = 512                 # q-quarter width
    NIT = NET * 4 * NKT      # 192 flat iterations

    x, wq, wk, wv, bq, bk, bv, mask, out = (
        aps["x"], aps["wq"], aps["wk"], aps["wv"],
        aps["bq"], aps["bk"], aps["bv"], aps["mask"], aps["out"],
    )

    stack = ExitStack()
    persist = stack.enter_context(tc.tile_pool(name="persist", bufs=1))
    xr_pool = stack.enter_context(tc.tile_pool(name="xr", bufs=4))
    wstage = stack.enter_context(tc.tile_pool(name="wst", bufs=1))
    sc_pool = stack.enter_context(tc.tile_pool(name="sc", bufs=2, space="PSUM"))
    ctx_pool = stack.enter_context(tc.tile_pool(name="ctx", bufs=2, space="PSUM"))
    misc_ps = stack.enter_context(tc.tile_pool(name="mps", bufs=2, space="PSUM"))
    et_pool = stack.enter_context(tc.tile_pool(name="et", bufs=6))
    r_pool = stack.enter_context(tc.tile_pool(name="r", bufs=3))
    r0_pool = stack.enter_context(tc.tile_pool(name="r0", bufs=3))
    rbc_pool = stack.enter_context(tc.tile_pool(name="rbc", bufs=3))
    oh_pool = stack.enter_context(tc.tile_pool(name="oh", bufs=4))

    # ---- DMA plan (sync queue, in order): X chunks 0-1, pair-0 e-tile of
    # Wq/Wk, X chunks 2-3, Wv, rest of Wq/Wk, X chunks 4-15.
    xrs = []

    def x_chunk_dma(c):
        t = xr_pool.tile([P, HIDDEN], f32, tag="xr", name=f"xr{c}")
        nc.sync.dma_start(out=t[:], in_=x[ts(c, P), :])
        xrs.append(t)

    w_parts = {}

    def w_dma(name, w, e0, e1, pname):
        t = wstage.tile([P, NDT, e1 - e0], f32, tag=pname, name=pname)
        nc.sync.dma_start(
            out=t[:], in_=w[:, e0:e1].rearrange("(t p) e -> p t e", p=P)
        )
        w_parts[pname] = (t, e0, e1)

    x_chunk_dma(0)
    x_chunk_dma(1)
    w_dma("q", wq, 0, P, "wq0")
    w_dma("k", wk, 0, P, "wk0")
    x_chunk_dma(2)
    x_chunk_dma(3)
    w_dma("v", wv, 0, ELOC, "wv")
    w_dma("q", wq, P, ELOC, "wqr")
    w_dma("k", wk, P, ELOC, "wkr")
    for c in range(4, NCH):
        x_chunk_dma(c)

    # ---- W casts f32->bf16 on DVE (pair-0 tiles first) ----
    w_sb = {}
    for name in ("q", "k", "v"):
        w_sb[name] = persist.tile([P, NDT, ELOC], bf16, tag=f"w{name}", name=f"w{name}")

    def w_cast(pname, dst):
        t, e0, e1 = w_parts[pname]
        nc.vector.tensor_copy(w_sb[dst][:, :, e0:e1], t[:])

    w_cast("wq0", "q")
    w_cast("wk0", "k")

    # ---- mask/bq/bk: load as rows (contiguous, descriptor-light, SWDGE),
    # then one PE transpose into per-partition layout.
    combo = persist.tile([32, P], f32, tag="combo")
    nc.gpsimd.memset(combo[:], 0.0)
    nc.gpsimd.dma_start(out=combo[0:NKT, :], in_=mask.rearrange("(t p) -> t p", p=P))
    nc.gpsimd.dma_start(out=combo[NKT : NKT + NET, :], in_=bq.rearrange("(t p) -> t p", p=P))
    nc.gpsimd.dma_start(out=combo[NKT + NET : NKT + 2 * NET, :], in_=bk.rearrange("(t p) -> t p", p=P))
    ident32 = persist.tile([32, 32], f32, tag="id32")
    make_identity(nc, ident32[:])
    const_ps = sc_pool.tile([P, 32], f32, tag="sc", name="constps")
    nc.tensor.transpose(const_ps[:], combo[:], ident32[:])
    const_sb = persist.tile([P, 32], f32, tag="const")
    nc.vector.tensor_copy(const_sb[:], const_ps[:])
    mask_sb = const_sb[:, 0:NKT]
    bq_sb = const_sb[:, NKT : NKT + NET]
    bk_sb = const_sb[:, NKT + NET : NKT + 2 * NET]

    bv_row = persist.tile([1, ELOC], f32, tag="bvr")
    nc.gpsimd.dma_start(out=bv_row[:], in_=bv[None, :])
    bv_bc = persist.tile([P, ELOC], f32, tag="bvb")
    nc.gpsimd.partition_broadcast(bv_bc[:], bv_row[:])

    # ---- X^T via PE transposes: chunk c -> 6 [128,128] transposes -> PSUM,
    # copy-cast f32->bf16 to xt (Scalar for the first chunks, DVE after) ----
    ident128 = persist.tile([P, P], f32, tag="id128")
    make_identity(nc, ident128[:])
    xt = persist.tile([P, NDT, S], bf16, tag="xt")

    def t_chunk(c):
        for g in range(2):
            ps = misc_ps.tile([P, 3, P], f32, tag="misc", name=f"tr{c}_{g}")
            for j in range(3):
                nc.tensor.transpose(ps[:, j, :], xrs[c][:, ts(3 * g + j, P)], ident128[:])
            dst = xt[:, 3 * g : 3 * g + 3, ts(c, P)]
            if c < 2:
                nc.scalar.copy(dst, ps[:])
            else:
                nc.vector.tensor_copy(dst, ps[:])

    # ---- V projection s-tile: V[s, e] = X @ Wv + bv, stored [128s, 6h, 65] bf16
    v_sb = persist.tile([P, NKT, HLOC, DHEAD + 1], bf16, tag="v")

    def v_tile(st):
        vps = misc_ps.tile([P, ELOC], f32, tag="misc", name=f"vps{st}")
        for dt_i in range(NDT):
            nc.tensor.matmul(
                vps[:],
                lhsT=xt[:, dt_i, ts(st, P)],
                rhs=w_sb["v"][:, dt_i, :],
                start=(dt_i == 0),
                stop=(dt_i == NDT - 1),
            )
        nc.vector.memset(v_sb[:, st, :, DHEAD:], 1.0)  # ones column
        nc.vector.tensor_add(
            v_sb[:, st, :, 0:DHEAD],
            vps[:].rearrange("p (h d) -> p h d", d=DHEAD),
            bv_bc[:].rearrange("p (h d) -> p h d", d=DHEAD),
        )

    # ---- Q^T / K^T projections: [e, s] = W.T @ X^T + b ----
    qt_sb = persist.tile([P, NET, S], bf16, tag="qt")
    kt_sb = persist.tile([P, NET, S], bf16, tag="kt")

    def qk_group(proj, et_i, sb_i):
        dst, b_sb = (qt_sb, bq_sb) if proj == "q" else (kt_sb, bk_sb)
        qps = misc_ps.tile([P, QQ], f32, tag="misc", name=f"qps{proj}{et_i}_{sb_i}")
        for dt_i in range(NDT):
            nc.tensor.matmul(
                qps[:],
                lhsT=w_sb[proj][:, dt_i, ts(et_i, P)],
                rhs=xt[:, dt_i, ts(sb_i, QQ)],
                start=(dt_i == 0),
                stop=(dt_i == NDT - 1),
            )
        nc.vector.tensor_scalar_add(
            dst[:, et_i, ts(sb_i, QQ)], qps[:], b_sb[:, et_i : et_i + 1]
        )

    # ---- background work, drained into the flat iteration stream ----
    bg = []

    def add_bg(due, fn):
        bg.append((due, len(bg), fn))

    for c in range(4, NCH):              # transposes for chunks 4-15
        add_bg(c - 4, lambda cc=c: t_chunk(cc))
    for st in range(NKT):                # V tiles; ctx(st) happens at it st+2
        add_bg(st, lambda tt=st: v_tile(tt))
    for sb in (1, 2, 3):                 # K(p0) k-chunks; scores need them at 4*sb
        add_bg(4 * sb - 1, lambda s=sb: qk_group("k", 0, s))
    for qq in (1, 2, 3):                 # Q(p0) q-chunks
        add_bg(16 * qq - 6, lambda q=qq: qk_group("q", 0, q))
    add_bg(18, lambda: w_cast("wqr", "q"))
    add_bg(19, lambda: w_cast("wkr", "k"))
    for pi, base in ((1, 22), (2, 86)):  # pair 1/2 projection prefetch
        jobs = [("q", 0), ("k", 0), ("k", 1), ("k", 2), ("k", 3),
                ("q", 1), ("q", 2), ("q", 3)]
        for j, (pr, sb) in enumerate(jobs):
            add_bg(base + 6 * j, lambda p=pr, e=pi, s=sb: qk_group(p, e, s))
    bg.sort(key=lambda e: (e[0], e[1]))
    bgi = [0]

    def drain(it):
        while bgi[0] < len(bg) and bg[bgi[0]][0] <= it:
            bg[bgi[0]][2]()
            bgi[0] += 1

    # ---- wv cast early (V(0) is needed at it 2); placed here so the DVE does
    # wq0/wk0/const/combo work first.
    w_cast("wv", "v")

    # ---- flat attention stream: iteration it = (pair, qq, k-tile t).
    # scores(it)+exp(it) each iteration; ctx(it-2) lags two iterations.
    ets = [None] * 6
    ctx_tiles = {}
    ohs_byp = {}

    def tail(p2, cq):
        ohs = ohs_byp[p2]
        tiles = ctx_tiles.pop((p2, cq))
        for hl in range(2):
            ctx_sb = r_pool.tile([DHEAD + 1, QQ], f32, tag="r")
            nc.vector.tensor_copy(ctx_sb[:], tiles[hl][:])
            r0 = r0_pool.tile([1, QQ], f32, tag="r0")
            nc.sync.dma_start(out=r0[:], in_=ctx_sb[DHEAD : DHEAD + 1, :])
            rr = r0_pool.tile([1, QQ], f32, tag="rr")
            nc.vector.reciprocal_approx_fast(rr[:], r0[:])
            rbc = rbc_pool.tile([DHEAD, QQ], f32, tag="rbc")
            nc.gpsimd.partition_broadcast(rbc[:], rr[:])
            nc.vector.tensor_mul(
                ohs[hl][:, ts(cq, QQ)], ctx_sb[0:DHEAD, :], rbc[:]
            )
            nc.sync.dma_start(
                out=out[ts(2 * p2 + hl, DHEAD), ts(cq, QQ)],
                in_=ohs[hl][:, ts(cq, QQ)],
            )

    def emit_ctx(ic):
        p2, rem = divmod(ic, 64)
        cq, ct = divmod(rem, 16)
        key = (p2, cq)
        if key not in ctx_tiles:
            ctx_tiles[key] = [
                ctx_pool.tile([DHEAD + 1, QQ], f32, tag="ctx", name=f"ctx{p2}_{cq}_{i}")
                for i in range(2)
            ]
        et_t = ets[ic % 6]
        for hl in range(2):
            nc.tensor.matmul(
                ctx_tiles[key][hl][:],
                lhsT=v_sb[:, ct, 2 * p2 + hl, :],
                rhs=et_t[:, hl, :],
                start=(ct == 0),
                stop=(ct == NKT - 1),
            )
        if ct == NKT - 1:
            tail(p2, cq)

    # pre-loop: first 4 chunks transposed, pair-0 first projections
    for c in range(4):
        t_chunk(c)
    qk_group("q", 0, 0)
    qk_group("k", 0, 0)

    for it in range(NIT):
        p2, rem = divmod(it, 64)
        qq, t = divmod(rem, 16)
        if p2 not in ohs_byp:
            ohs_byp[p2] = [
                oh_pool.tile([DHEAD, S], f32, tag="oh", name=f"oh{p2}_{i}")
                for i in range(2)
            ]
        if it >= 2:
            emit_ctx(it - 2)
        s_t = sc_pool.tile([P, 2, QQ], f32, tag="sc", name=f"s{it}")
        for hl in range(2):
            rows = slice(DHEAD * hl, DHEAD * (hl + 1))
            nc.tensor.matmul(
                s_t[:, hl, :],
                lhsT=kt_sb[rows, p2, ts(t, P)],
                rhs=qt_sb[rows, p2, ts(qq, QQ)],
                start=True,
                stop=True,
                tile_position=(DHEAD * hl, 0),
            )
        et_t = et_pool.tile([P, 2, QQ], bf16, tag="et", name=f"et{it}")
        ets[it % 6] = et_t
        nc.scalar.activation(
            et_t[:], s_t[:], Exp,
            bias=mask_sb[:, t : t + 1], scale=0.125,
        )
        drain(it)
    emit_ctx(NIT - 2)
    emit_ctx(NIT - 1)
    drain(10 ** 9)

    stack.close()


def build():
    """Build and compile the per-core Bass program (same program on all 8 cores)."""
    if "nc" in _CACHE:
        return _CACHE["nc"]
    import concourse.bass as bass  # noqa: F401
    import concourse.tile as tile
    from concourse import bacc, mybir

    f32 = mybir.dt.float32
    nc = bacc.Bacc("TRN2", target_bir_lowering=False, debug=False, num_devices=NCORES)
    aps = {
        "x": nc.dram_tensor("x", [S, HIDDEN], f32, kind="ExternalInput").ap(),
        "wq": nc.dram_tensor("wq", [HIDDEN, ELOC], f32, kind="ExternalInput").ap(),
        "wk": nc.dram_tensor("wk", [HIDDEN, ELOC], f32, kind="ExternalInput").ap(),
        "wv": nc.dram_tensor("wv", [HIDDEN, ELOC], f32, kind="ExternalInput").ap(),
        "bq": nc.dram_tensor("bq", [ELOC], f32, kind="ExternalInput").ap(),
        "bk": nc.dram_tensor("bk", [ELOC], f32, kind="ExternalInput").ap(),
        "bv": nc.dram_tensor("bv", [ELOC], f32, kind="ExternalInput").ap(),
        "mask": nc.dram_tensor("mask", [S], f32, kind="ExternalInput").ap(),
        "out": nc.dram_tensor("out", [ELOC, S], f32, kind="ExternalOutput").ap(),
    }
    with tile.TileContext(nc) as tc:
        _emit(tc, aps)
    nc.compile()
    _CACHE["nc"] = nc
    return nc


def shard_inputs(hidden_states, attention_mask, Wq, bq, Wk, bk, Wv, bv):
    in_maps = []
    for c in range(NCORES):
        b, g = divmod(c, 2)
        cols = slice(ELOC * g, ELOC * (g + 1))
        in_maps.append({
            "x": np.ascontiguousarray(hidden_states[b], dtype=np.float32),
            "wq": np.ascontiguousarray(Wq[:, cols], dtype=np.float32),
            "wk": np.ascontiguousarray(Wk[:, cols], dtype=np.float32),
            "wv": np.ascontiguousarray(Wv[:, cols], dtype=np.float32),
            "bq": np.ascontiguousarray(bq[cols], dtype=np.float32),
            "bk": np.ascontiguousarray(bk[cols], dtype=np.float32),
            "bv": np.ascontiguousarray(bv[cols], dtype=np.float32),
            "mask": np.ascontiguousarray(
                np.asarray(attention_mask, dtype=np.float32)[b].reshape(S)
            ),
        })
    return in_maps


def gather_outputs(results):
    out = np.empty((B, S, HIDDEN), dtype=np.float32)
    for c in range(NCORES):
        b, g = divmod(c, 2)
        out[b, :, ELOC * g : ELOC * (g + 1)] = np.ascontiguousarray(results[c]["out"].T)
    return out


def kernel(**inputs):
    from concourse.bass_utils import run_bass_kernel_spmd

    nc = build()
    in_maps = shard_inputs(**{k: np.asarray(v) for k, v in inputs.items()})
    res = run_bass_kernel_spmd(nc, in_maps, list(range(NCORES)))
    return gather_outputs(res.results)


if __name__ == "__main__":
    nc = build()
    print("build + compile OK")
